# revision 34
# baseline (speedup 1.0000x reference)
"""Known-good v3 fallback: 296,652 ns, rel err 1.319e-02 on the harness.

fp16 pipeline, head-paired row-tiled QK, swapped PV (outT), exp split
ACT/DVE through the shared psum_qk rotation, PE-transpose finalize.
"""

import numpy as np

P = 128
E = 64
NH = 2


def _build(L=4096, S=4096, LT=512, n_dve=8, at_bufs=2, num_devices=8):
    import concourse.mybir as mybir
    import concourse.tile as tile
    from concourse import bacc
    from concourse.masks import make_identity

    f32 = mybir.dt.float32
    f16 = mybir.dt.float16
    i16 = mybir.dt.int16
    Exp = mybir.ActivationFunctionType.Exp
    Alu = mybir.AluOpType

    NS = S // P
    LT = min(LT, L)
    NLT = L // LT
    NLS = LT // P
    scale = float(E) ** -0.5

    A_s = scale * (2.0 ** 10) / float(np.log(2.0))
    B_s = 15.0 * 2 ** 10 - 58.4
    if n_dve:
        base = min(8, NS // 2)
        dve_set = {base + (i * (NS - base)) // n_dve for i in range(n_dve)}
    else:
        dve_set = set()

    nc = bacc.Bacc(
        "TRN2", target_bir_lowering=False, debug=False, num_devices=num_devices
    )
    q = nc.dram_tensor("q", [L, NH, E], f32, kind="ExternalInput").ap()
    k = nc.dram_tensor("k", [S, NH, E], f32, kind="ExternalInput").ap()
    v = nc.dram_tensor("v", [S, NH, E], f32, kind="ExternalInput").ap()
    o = nc.dram_tensor("o", [L, NH, E], f32, kind="ExternalOutput").ap()

    with tile.TileContext(nc) as tc:
        with (
            tc.tile_pool(name="persist", bufs=1) as persist,
            tc.tile_pool(name="stage_q", bufs=4) as stage_q,
            tc.tile_pool(name="stage_k", bufs=4) as stage_k,
            tc.tile_pool(name="stage_v", bufs=4) as stage_v,
            tc.tile_pool(name="stageb", bufs=4) as stageb,
            tc.tile_pool(name="attn", bufs=at_bufs) as attn_pool,
            tc.tile_pool(name="outp", bufs=4) as outp,
            tc.tile_pool(name="obuf", bufs=2) as obuf,
            tc.tile_pool(name="psum_qk", bufs=2, space="PSUM") as psum_qk,
            tc.tile_pool(name="psum_pv", bufs=2, space="PSUM") as psum_pv,
            tc.tile_pool(name="psum_tr", bufs=2, space="PSUM") as psum_tr,
        ):
            identh = persist.tile([P, P], f16, name="identh")
            make_identity(nc, identh)

            kT2 = persist.tile([P, S], f16, name="kT2")
            qT2 = persist.tile([P, L], f16, name="qT2")
            vx = persist.tile([P, NH, NS, P], f16, name="vx")
            nc.gpsimd.memset(vx[:], 0.0)
            nc.gpsimd.memset(vx[:, :, :, E : E + 1], 1.0)

            def load_q(c):
                qc = stage_q.tile([P, NH, E], f32, name="qc")
                nc.sync.dma_start(qc[:], q[c * P : (c + 1) * P, :, :])
                qcb = stageb.tile([P, NH, E], f16, name="qcb")
                nc.vector.tensor_copy(qcb[:], qc[:])
                pq = psum_tr.tile([P, P], f16, name="ptr")
                nc.tensor.transpose(pq[:], qcb[:], identh)
                nc.vector.tensor_copy(qT2[:, c * P : (c + 1) * P], pq[:])

            def load_k(c):
                kc = stage_k.tile([P, NH, E], f32, name="kc")
                nc.sync.dma_start(kc[:], k[c * P : (c + 1) * P, :, :])
                kcb = stageb.tile([P, NH, E], f16, name="kcb")
                nc.vector.tensor_copy(kcb[:], kc[:])
                pk = psum_tr.tile([P, P], f16, name="ptr")
                nc.tensor.transpose(pk[:], kcb[:], identh)
                nc.vector.tensor_copy(kT2[:, c * P : (c + 1) * P], pk[:])

            def load_v(c):
                vc = stage_v.tile([P, NH, E], f32, name="vc")
                nc.sync.dma_start(vc[:], v[c * P : (c + 1) * P, :, :])
                nc.vector.tensor_copy(vx[:, :, c, :E], vc[:])

            for c in range(min(4, NS)):
                load_q(c)
            for c in range(NS):
                load_k(c)
            for c in range(NS):
                load_v(c)
                if c + 4 < NS:
                    load_q(c + 4)

            for lt in range(NLT):
                l0 = lt * LT
                at = attn_pool.tile([P, NS, NH, LT], f16, name="at")
                for c in range(NS):
                    ps = psum_qk.tile([P, NH, LT], f32, name="ps")
                    for h in range(NH):
                        nc.tensor.matmul(
                            ps[:, h, :],
                            lhsT=kT2[h * E : (h + 1) * E, c * P : (c + 1) * P],
                            rhs=qT2[h * E : (h + 1) * E, l0 : l0 + LT],
                            start=True,
                            stop=True,
                        )
                    if c in dve_set:
                        nc.vector.tensor_scalar(
                            at[:, c, :, :].bitcast(i16),
                            ps[:, :, :],
                            A_s,
                            B_s,
                            op0=Alu.mult,
                            op1=Alu.add,
                        )
                    else:
                        nc.scalar.activation(
                            at[:, c, :, :], ps[:, :, :], Exp, scale=scale
                        )

                pv = [
                    psum_pv.tile([P, LT], f32, name="pv") for _ in range(NH)
                ]
                for c in range(NS):
                    for h in range(NH):
                        nc.tensor.matmul(
                            pv[h][:, :],
                            lhsT=vx[:, h, c, :],
                            rhs=at[:, c, h, :],
                            start=(c == 0),
                            stop=(c == NS - 1),
                        )
                for h in range(NH):
                    pvc = obuf.tile([E + 1, LT], f16, name="pvc")
                    nc.vector.tensor_copy(pvc[:], pv[h][: E + 1, :])
                    for m in range(NLS):
                        tp = psum_tr.tile([P, E + 1], f16, name="ptr")
                        nc.tensor.transpose(
                            tp[:],
                            pvc[:, m * P : (m + 1) * P],
                            identh[: E + 1, : E + 1],
                        )
                        rec = outp.tile([P, 1], f32, name="rec")
                        nc.vector.reciprocal(rec[:], tp[:, E : E + 1])
                        ot = outp.tile([P, E], f32, name="ot")
                        nc.vector.tensor_scalar_mul(ot[:], tp[:, :E], rec[:])
                        nc.sync.dma_start(
                            o[l0 + m * P : l0 + (m + 1) * P, h, :], ot[:]
                        )

    nc.compile()
    return nc


_CACHE = {}


def _get_nc():
    if "nc" not in _CACHE:
        _CACHE["nc"] = _build()
    return _CACHE["nc"]


def kernel(q, k, v):
    from concourse.bass_utils import run_bass_kernel_spmd

    q = np.asarray(q)
    k = np.asarray(k)
    v = np.asarray(v)
    B, L, H, _E = q.shape

    nc = _get_nc()
    in_maps = []
    for c in range(8):
        b, hq = divmod(c, 4)
        h0 = hq * NH
        in_maps.append(
            {
                "q": np.ascontiguousarray(q[b, :, h0 : h0 + NH, :]),
                "k": np.ascontiguousarray(k[b, :, h0 : h0 + NH, :]),
                "v": np.ascontiguousarray(v[b, :, h0 : h0 + NH, :]),
            }
        )
    res = run_bass_kernel_spmd(nc, in_maps, list(range(8)))
    out = np.empty((B, L, H, _E), np.float32)
    for c in range(8):
        b, hq = divmod(c, 4)
        h0 = hq * NH
        out[b, :, h0 : h0 + NH, :] = res.results[c]["o"]
    return out


# revision 37
# speedup vs baseline: 1.1261x; 1.1261x over previous
"""Known-good v3 fallback: 296,652 ns, rel err 1.319e-02 on the harness.

fp16 pipeline, head-paired row-tiled QK, swapped PV (outT), exp split
ACT/DVE through the shared psum_qk rotation, PE-transpose finalize.
"""

import numpy as np

P = 128
E = 64
NH = 2


def _build(L=4096, S=4096, LT=512, n_dve=8, at_bufs=2, num_devices=8):
    import concourse.mybir as mybir
    import concourse.tile as tile
    from concourse import bacc
    from concourse.masks import make_identity

    f32 = mybir.dt.float32
    f16 = mybir.dt.float16
    i16 = mybir.dt.int16
    Exp = mybir.ActivationFunctionType.Exp
    Alu = mybir.AluOpType

    NS = S // P
    LT = min(LT, L)
    NLT = L // LT
    NLS = LT // P
    scale = float(E) ** -0.5

    A_s = scale * (2.0 ** 10) / float(np.log(2.0))
    B_s = 15.0 * 2 ** 10 - 58.4
    if n_dve:
        base = min(8, NS // 2)
        dve_set = {base + (i * (NS - base)) // n_dve for i in range(n_dve)}
    else:
        dve_set = set()

    nc = bacc.Bacc(
        "TRN2", target_bir_lowering=False, debug=False, num_devices=num_devices
    )
    q = nc.dram_tensor("q", [L, NH, E], f32, kind="ExternalInput").ap()
    k = nc.dram_tensor("k", [S, NH, E], f32, kind="ExternalInput").ap()
    v = nc.dram_tensor("v", [S, NH, E], f32, kind="ExternalInput").ap()
    o = nc.dram_tensor("o", [L, NH, E], f32, kind="ExternalOutput").ap()

    with tile.TileContext(nc) as tc:
        with (
            tc.tile_pool(name="persist", bufs=1) as persist,
            tc.tile_pool(name="stage_q", bufs=4) as stage_q,
            tc.tile_pool(name="stage_k", bufs=4) as stage_k,
            tc.tile_pool(name="stage_v", bufs=4) as stage_v,
            tc.tile_pool(name="stageb", bufs=4) as stageb,
            tc.tile_pool(name="attn", bufs=at_bufs) as attn_pool,
            tc.tile_pool(name="outp", bufs=4) as outp,
            tc.tile_pool(name="obuf", bufs=2) as obuf,
            tc.tile_pool(name="psum_qk", bufs=2, space="PSUM") as psum_qk,
            tc.tile_pool(name="psum_pv", bufs=2, space="PSUM") as psum_pv,
            tc.tile_pool(name="psum_tr", bufs=2, space="PSUM") as psum_tr,
        ):
            identh = persist.tile([P, P], f16, name="identh")
            # Warm up the Vector engine before anything depends on it (the
            # first DVE op after engine init measures ~5us).
            warm = persist.tile([P, 8], f16, name="warm")
            nc.vector.memset(warm[:], 0.0)
            nc.vector.tensor_copy(warm[:, :4], warm[:, 4:])
            make_identity(nc, identh)

            kT2 = persist.tile([P, S], f16, name="kT2")
            qT2 = persist.tile([P, L], f16, name="qT2")
            vx = persist.tile([P, NH, NS, P], f16, name="vx")
            nc.gpsimd.memset(vx[:], 0.0)
            nc.gpsimd.memset(vx[:, :, :, E : E + 1], 1.0)

            def load_q(c):
                qc = stage_q.tile([P, NH, E], f32, name="qc")
                nc.sync.dma_start(qc[:], q[c * P : (c + 1) * P, :, :])
                qcb = stageb.tile([P, NH, E], f16, name="qcb")
                nc.vector.tensor_copy(qcb[:], qc[:])
                pq = psum_tr.tile([P, P], f16, name="ptr")
                nc.tensor.transpose(pq[:], qcb[:], identh)
                nc.vector.tensor_copy(qT2[:, c * P : (c + 1) * P], pq[:])

            def load_k(c):
                kc = stage_k.tile([P, NH, E], f32, name="kc")
                nc.sync.dma_start(kc[:], k[c * P : (c + 1) * P, :, :])
                kcb = stageb.tile([P, NH, E], f16, name="kcb")
                nc.vector.tensor_copy(kcb[:], kc[:])
                pk = psum_tr.tile([P, P], f16, name="ptr")
                nc.tensor.transpose(pk[:], kcb[:], identh)
                nc.vector.tensor_copy(kT2[:, c * P : (c + 1) * P], pk[:])

            def load_v(c):
                vc = stage_v.tile([P, NH, E], f32, name="vc")
                nc.sync.dma_start(vc[:], v[c * P : (c + 1) * P, :, :])
                nc.vector.tensor_copy(vx[:, :, c, :E], vc[:])

            for c in range(min(4, NS)):
                load_q(c)
            for c in range(NS):
                load_k(c)
            for c in range(NS):
                load_v(c)
                if c + 4 < NS:
                    load_q(c + 4)

            for lt in range(NLT):
                l0 = lt * LT
                at = attn_pool.tile([P, NS, NH, LT], f16, name="at")
                for c in range(NS):
                    # DVE chunks use slots from the psum_tr pool so the ACT
                    # exp chain's psum_qk rotation never waits on the
                    # Vector engine (decouples the two exp streams). In lt
                    # 0 only late chunks qualify — early ones would queue
                    # behind still-running phase-A transposes in the same
                    # pool rotation and stall the in-order PE queue.
                    if (lt > 0 or c >= 24) and c in dve_set:
                        for h in range(NH):
                            pd = psum_tr.tile([P, LT], f32, name="ptr")
                            nc.tensor.matmul(
                                pd[:, :],
                                lhsT=kT2[
                                    h * E : (h + 1) * E, c * P : (c + 1) * P
                                ],
                                rhs=qT2[h * E : (h + 1) * E, l0 : l0 + LT],
                                start=True,
                                stop=True,
                            )
                            nc.vector.tensor_scalar(
                                at[:, c, h, :].bitcast(i16),
                                pd[:, :],
                                A_s,
                                B_s,
                                op0=Alu.mult,
                                op1=Alu.add,
                            )
                        continue
                    ps = psum_qk.tile([P, NH, LT], f32, name="ps")
                    for h in range(NH):
                        nc.tensor.matmul(
                            ps[:, h, :],
                            lhsT=kT2[h * E : (h + 1) * E, c * P : (c + 1) * P],
                            rhs=qT2[h * E : (h + 1) * E, l0 : l0 + LT],
                            start=True,
                            stop=True,
                        )
                    nc.scalar.activation(
                        at[:, c, :, :], ps[:, :, :], Exp, scale=scale
                    )

                pv = [
                    psum_pv.tile([P, LT], f32, name="pv") for _ in range(NH)
                ]
                for c in range(NS):
                    for h in range(NH):
                        nc.tensor.matmul(
                            pv[h][:, :],
                            lhsT=vx[:, h, c, :],
                            rhs=at[:, c, h, :],
                            start=(c == 0),
                            stop=(c == NS - 1),
                        )
                # Finalize with the two heads' chains interleaved per
                # l-subtile so PE transposes, DVE rec/mul and stores
                # pipeline across heads (shortens the kernel tail).
                pvcs = []
                for h in range(NH):
                    pvc = obuf.tile([E + 1, LT], f16, name="pvc")
                    nc.vector.tensor_copy(pvc[:], pv[h][: E + 1, :])
                    pvcs.append(pvc)
                for m in range(NLS):
                    for h in range(NH):
                        tp = psum_tr.tile([P, E + 1], f16, name="ptr")
                        nc.tensor.transpose(
                            tp[:],
                            pvcs[h][:, m * P : (m + 1) * P],
                            identh[: E + 1, : E + 1],
                        )
                        rec = outp.tile([P, 1], f32, name="rec")
                        nc.vector.reciprocal(rec[:], tp[:, E : E + 1])
                        ot = outp.tile([P, E], f32, name="ot")
                        nc.vector.tensor_scalar_mul(ot[:], tp[:, :E], rec[:])
                        nc.sync.dma_start(
                            o[l0 + m * P : l0 + (m + 1) * P, h, :], ot[:]
                        )

    nc.compile()
    return nc


_CACHE = {}


def _get_nc():
    if "nc" not in _CACHE:
        _CACHE["nc"] = _build()
    return _CACHE["nc"]


def kernel(q, k, v):
    from concourse.bass_utils import run_bass_kernel_spmd

    q = np.asarray(q)
    k = np.asarray(k)
    v = np.asarray(v)
    B, L, H, _E = q.shape

    nc = _get_nc()
    in_maps = []
    for c in range(8):
        b, hq = divmod(c, 4)
        h0 = hq * NH
        in_maps.append(
            {
                "q": np.ascontiguousarray(q[b, :, h0 : h0 + NH, :]),
                "k": np.ascontiguousarray(k[b, :, h0 : h0 + NH, :]),
                "v": np.ascontiguousarray(v[b, :, h0 : h0 + NH, :]),
            }
        )
    res = run_bass_kernel_spmd(nc, in_maps, list(range(8)))
    out = np.empty((B, L, H, _E), np.float32)
    for c in range(8):
        b, hq = divmod(c, 4)
        h0 = hq * NH
        out[b, :, h0 : h0 + NH, :] = res.results[c]["o"]
    return out
